# revision 5
# baseline (speedup 1.0000x reference)
"""Causal self-attention (B=4, T=2048, C=1024, H=16) on 8 trn2 NeuronCores.

Sharding: core c -> batch b = c//2, heads h0 = (c%2)*8 .. h0+8 (tensor
parallel over heads: c_attn columns / c_proj rows split). Each core computes a
partial projection output [T, C]; the host sums the two partials per batch and
adds b_proj.

Device-side dataflow (bf16 matmul data, fp32 accumulation):
  - host passes x[b] pre-transposed as xt [C, T]
  - qT [C_head, T] computed with W_q column-slices stationary; bias bq added
    on DVE. kT has NO bias: softmax over keys is invariant to q.bk and bq.bk
    terms, so only (q+bq).k is needed -> k = x@W_k raw (saves ScalarE work)
  - v in natural [T, D] layout + a ones column so the PV matmul also produces
    the softmax denominator (row 64 of the PV accumulator)
  - scores: per head PAIR (2m, 2m+1), ROW-TILED 64-contraction matmuls at
    tile_position (0,0)/(64,0) run concurrently (~2x PE throughput); the
    sibling partition halves of kT/qT hold the two heads, no zero padding
  - attention runs per (pair, q-phase of 1024): PSUM = 2 x psS[128,1024]
    + 2 x psY[65,1024] = exactly 8 banks
  - P~ = exp(S^T/8) on ScalarE (the ONLY ScalarE work; scores are O(1) so no
    max-subtraction); diagonal 128x128 blocks masked with upper-tri 0/1 mask
  - normalize: DVE reciprocal of PSUM row 64 -> gpsimd partition_broadcast ->
    DVE multiply writing bf16 yT (no DRAM round-trips)
  - proj: bf16 stationary yT tiles (FWL) x bf16 W_proj rows, PSUM -> SBUF via
    DVE, DMA out on sync/gpsimd queues
"""

import numpy as np

P = 128


def _bf16_np():
    import ml_dtypes
    return ml_dtypes.bfloat16


def build_program(T=2048, C=1024, HC=8, D=64, num_devices=8, trn="TRN2"):
    import concourse.mybir as mybir
    import concourse.tile as tile
    from concourse import bacc
    from concourse.masks import make_upper_triangular

    W = 512          # matmul moving-dim chunk (psum-bank limit for fp32 out)
    PH = 1024        # attention q-phase width
    KC = C // P      # contraction tiles over C
    CO = HC * D      # this core's qkv channel block (512)
    NP = CO // P     # head pairs (2 heads of 64 = 1 partition tile)
    TT = T // P      # T tiles
    NCH = T // W     # T chunks
    MV = D + 1       # PV stationary columns: v + ones
    WO = 512         # proj output column chunk
    NW = C // WO
    dt32 = mybir.dt.float32
    bf16 = mybir.dt.bfloat16
    ActF = mybir.ActivationFunctionType
    Alu = mybir.AluOpType
    scale = 1.0 / float(np.sqrt(D))

    nc = bacc.Bacc(trn, target_bir_lowering=False, debug=False,
                   enable_asserts=False, num_devices=num_devices)

    xt_d = nc.dram_tensor("xt", [C, T], bf16, kind="ExternalInput")
    wq_d = nc.dram_tensor("wq", [C, CO], bf16, kind="ExternalInput")
    wk_d = nc.dram_tensor("wk", [C, CO], bf16, kind="ExternalInput")
    wv_d = nc.dram_tensor("wv", [C, CO], bf16, kind="ExternalInput")
    bq_d = nc.dram_tensor("bq", [P, NP], dt32, kind="ExternalInput")
    bvb_d = nc.dram_tensor("bvb", [P, CO], dt32, kind="ExternalInput")
    ones_d = nc.dram_tensor("ones", [P, TT * HC], bf16, kind="ExternalInput")
    wp_d = nc.dram_tensor("wp", [CO, C], bf16, kind="ExternalInput")
    out_d = nc.dram_tensor("out", [T, C], dt32, kind="ExternalOutput")

    with tile.TileContext(nc) as tc:
        with tc.tile_pool(name="const", bufs=1) as cpool, \
             tc.tile_pool(name="pers", bufs=1) as pers:
            tri = cpool.tile([P, P], bf16)
            make_upper_triangular(nc, tri[:], val=1.0, diag=True)
            bq_sb = cpool.tile([P, NP], dt32)
            nc.sync.dma_start(bq_sb[:], bq_d.ap())
            bvb_sb = cpool.tile([P, CO], dt32)
            nc.sync.dma_start(bvb_sb[:], bvb_d.ap())

            qT = pers.tile([P, NP, T], bf16, tag="qT")
            kT = pers.tile([P, HC, T], bf16, tag="kT")
            vaug = pers.tile([P, TT, HC, MV], bf16, tag="vaug")
            nc.sync.dma_start(
                vaug[:, :, :, D],
                ones_d.ap().rearrange("p (a b) -> p a b", b=HC))

            # ---------------- stage B: qkv projections ----------------
            with nc.named_scope("qkv"), \
                 tc.tile_pool(name="xtp", bufs=KC * NCH) as xpool, \
                 tc.tile_pool(name="wp_in", bufs=KC) as wpool, \
                 tc.tile_pool(name="psB", bufs=2, space="PSUM") as psB:
                xt_view = xt_d.ap().rearrange("(kc p) t -> kc p t", p=P)
                wq_view = wq_d.ap().rearrange("(kc p) n -> kc p n", p=P)
                xts = []
                wq_t = []
                dmae = [nc.sync, nc.gpsimd]
                di = 0
                for kc in range(KC):
                    wt = wpool.tile([P, CO], bf16, tag="w")
                    dmae[di % 2].dma_start(wt[:], wq_view[kc])
                    di += 1
                    wq_t.append(wt)
                    row = []
                    for cg in range(NCH):
                        xtc = xpool.tile([P, W], bf16, tag="xt")
                        dmae[di % 2].dma_start(
                            xtc[:], xt_view[kc][:, cg * W:(cg + 1) * W])
                        di += 1
                        row.append(xtc)
                    xts.append(row)

                def qk_stage(w_tiles, write_out):
                    for half in range((NP + 1) // 2):
                        ms = [m for m in (2 * half, 2 * half + 1) if m < NP]
                        pss = {}
                        for m in ms:
                            ps_m = psB.tile([P, T], dt32, tag="psB")
                            pss[m] = ps_m
                        for kc in range(KC):
                            for m in ms:
                                for cg in range(NCH):
                                    nc.tensor.matmul(
                                        pss[m][:, cg * W:(cg + 1) * W],
                                        w_tiles[kc][:, m * P:(m + 1) * P],
                                        xts[kc][cg][:],
                                        start=(kc == 0), stop=(kc == KC - 1),
                                        skip_group_check=True)
                        for m in ms:
                            write_out(m, pss[m])

                def write_qT(m, ps):
                    nc.vector.tensor_scalar_add(
                        qT[:, m, :], ps[:], bq_sb[:, m:m + 1])

                def write_kT(m, ps):
                    # unpacked per-head halves (row-tiled scores never read
                    # the sibling half, so no zero fill needed)
                    nc.vector.tensor_copy(kT[0:D, 2 * m, :], ps[0:D, :])
                    nc.vector.tensor_copy(kT[D:P, 2 * m + 1, :], ps[D:P, :])

                def load_w(w_d):
                    view = w_d.ap().rearrange("(kc p) n -> kc p n", p=P)
                    tiles = []
                    for kc in range(KC):
                        wt = wpool.tile([P, CO], bf16, tag="w")
                        dmae[kc % 2].dma_start(wt[:], view[kc])
                        tiles.append(wt)
                    return tiles

                qk_stage(wq_t, write_qT)
                qk_stage(load_w(wk_d), write_kT)

                wv_t = load_w(wv_d)
                bvb_v = bvb_sb[:].rearrange("p (h d) -> p h d", d=D)
                for tt in range(TT):
                    ps = psB.tile([P, CO], dt32, tag="psB")
                    for kc in range(KC):
                        nc.tensor.matmul(
                            ps[:],
                            xts[kc][tt * P // W][:, (tt * P % W):(tt * P % W) + P],
                            wv_t[kc][:],
                            start=(kc == 0), stop=(kc == KC - 1))
                    nc.vector.scalar_tensor_tensor(
                        out=vaug[:, tt, :, 0:D],
                        in0=ps[:].rearrange("p (h d) -> p h d", d=D),
                        scalar=1.0, in1=bvb_v,
                        op0=Alu.mult, op1=Alu.add)

            # ---------------- stage C: attention per head pair ----------------
            late_cm = tc.tile_pool(name="late", bufs=1)
            late = late_cm.__enter__()
            yT = late.tile([P, NP, T], bf16, tag="yT")
            with nc.named_scope("attn"), \
                 tc.tile_pool(name="ptp", bufs=4) as ptpool, \
                 tc.tile_pool(name="nrm", bufs=2) as nrmpool, \
                 tc.tile_pool(name="psS", bufs=2, space="PSUM") as psS, \
                 tc.tile_pool(name="psY", bufs=2, space="PSUM") as psY:

                def emit_s(h, ti, m, j, q0):
                    """Row-tiled scores for head h (row half ti) + exp -> pt."""
                    jb = j * P
                    qs0 = max(jb, q0)
                    qe_ph = q0 + PH
                    rows = slice(ti * D, ti * D + D)
                    sps = psS.tile([P, PH], dt32, tag="s")
                    for cg in range(qs0 // W, qe_ph // W):
                        qs = max(W * cg, qs0)
                        w = W * (cg + 1) - qs
                        nc.tensor.matmul(
                            sps[:, qs - q0:qs - q0 + w],
                            kT[rows, h, jb:jb + P],
                            qT[rows, m, qs:qs + w],
                            start=True, stop=True,
                            skip_group_check=True)
                    pt = ptpool.tile([P, qe_ph - qs0], bf16, tag="pt")
                    nc.scalar.activation(
                        pt[:], sps[:, qs0 - q0:PH], ActF.Exp, scale=scale)
                    if jb >= q0:  # diagonal block: mask upper triangle
                        nc.vector.tensor_mul(pt[:, 0:P], pt[:, 0:P], tri[:])
                    return pt

                def emit_pv(yt, h, j, q0, pt):
                    jb = j * P
                    qs0 = max(jb, q0)
                    for cg in range(qs0 // W, (q0 + PH) // W):
                        qs = max(W * cg, qs0)
                        w = W * (cg + 1) - qs
                        last_j = (W * (cg + 1)) // P - 1
                        nc.tensor.matmul(
                            yt[:, qs - q0:qs - q0 + w],
                            vaug[:, j, h, :],
                            pt[:, qs - qs0:qs - qs0 + w],
                            start=(j == 0), stop=(j == last_j),
                            skip_group_check=True)

                def finish(yt, h, m, q0):
                    r0 = (h % 2) * D
                    lrow = nrmpool.tile([1, PH], dt32, tag="lrow")
                    nc.vector.reciprocal(lrow[:], yt[D:D + 1, :])
                    bc = nrmpool.tile([D, PH], dt32, tag="bc")
                    nc.gpsimd.partition_broadcast(bc[:], lrow[:])
                    nc.vector.tensor_mul(
                        yT[r0:r0 + D, m, q0:q0 + PH], yt[0:D, :], bc[:])

                for m in range(NP):
                    heads = (2 * m, 2 * m + 1)
                    for q0 in (0, PH):
                        js = range((q0 + PH) // P)
                        yts = {}
                        for ti, h in enumerate(heads):
                            yt_new = psY.tile([MV, PH], dt32, tag="yt")
                            yts[h] = yt_new
                        pending = []
                        for j in js:
                            for ti, h in enumerate(heads):
                                pt = emit_s(h, ti, m, j, q0)
                                pending.append((h, j, pt))
                                if len(pending) > 2:
                                    ph, pj, ppt = pending.pop(0)
                                    emit_pv(yts[ph], ph, pj, q0, ppt)
                        for ph, pj, ppt in pending:
                            emit_pv(yts[ph], ph, pj, q0, ppt)
                        for ti, h in enumerate(heads):
                            finish(yts[h], h, m, q0)

            # ---------------- stage E: output projection ----------------
            with nc.named_scope("proj"), \
                 tc.tile_pool(name="wpp", bufs=1) as wppool, \
                 tc.tile_pool(name="ost", bufs=3) as opool, \
                 tc.tile_pool(name="psO", bufs=4, space="PSUM") as psO:
                wpsb = wppool.tile([P, NP, C], bf16)
                nc.sync.dma_start(
                    wpsb[:], wp_d.ap().rearrange("(kt p) n -> p kt n", p=P))
                for tt in range(TT):
                    po = psO.tile([P, C], dt32, tag="o")
                    for kt in range(NP):
                        for nn in range(NW):
                            nc.tensor.matmul(
                                po[:, nn * WO:(nn + 1) * WO],
                                yT[:, kt, tt * P:(tt + 1) * P],
                                wpsb[:, kt, nn * WO:(nn + 1) * WO],
                                start=(kt == 0), stop=(kt == NP - 1),
                                skip_group_check=True)
                    ot = opool.tile([P, C], dt32, tag="ot")
                    nc.vector.tensor_copy(ot[:], po[:])
                    [nc.sync, nc.gpsimd][tt % 2].dma_start(
                        out_d.ap()[tt * P:(tt + 1) * P, :], ot[:])
            late_cm.__exit__(None, None, None)

    nc.compile()
    return nc


def make_core_inputs(x, W_attn, b_attn, W_proj, n_cores=8, HC=8, D=64):
    """Host-side sharding: per-core input dicts."""
    B, T, C = x.shape
    CO = HC * D
    NP = CO // P
    bf = _bf16_np()
    in_maps = []
    for c in range(n_cores):
        b = c // (n_cores // B)
        h0 = (c % (n_cores // B)) * HC
        lo = h0 * D
        bq = b_attn[lo:lo + CO]
        bv = b_attn[2 * C + lo:2 * C + lo + CO]
        in_maps.append({
            "xt": np.ascontiguousarray(x[b].T).astype(bf),
            "wq": np.ascontiguousarray(W_attn[:, lo:lo + CO]).astype(bf),
            "wk": np.ascontiguousarray(W_attn[:, C + lo:C + lo + CO]).astype(bf),
            "wv": np.ascontiguousarray(W_attn[:, 2 * C + lo:2 * C + lo + CO]).astype(bf),
            "bq": np.ascontiguousarray(bq.reshape(NP, P).T),
            "bvb": np.tile(bv[None, :], (P, 1)),
            "ones": np.ones((P, (T // P) * HC), bf),
            "wp": np.ascontiguousarray(W_proj[lo:lo + CO, :]).astype(bf),
        })
    return in_maps


_CACHE = {}


def _get_program():
    if "nc" not in _CACHE:
        _CACHE["nc"] = build_program()
    return _CACHE["nc"]


def run_on_cores(x, W_attn, b_attn, W_proj, b_proj, trace=False):
    """Returns (full output [B,T,C], BassKernelResults)."""
    from concourse.bass_utils import run_bass_kernel_spmd

    x = np.asarray(x, np.float32)
    W_attn = np.asarray(W_attn, np.float32)
    b_attn = np.asarray(b_attn, np.float32)
    W_proj = np.asarray(W_proj, np.float32)
    b_proj = np.asarray(b_proj, np.float32)

    nc = _get_program()
    in_maps = make_core_inputs(x, W_attn, b_attn, W_proj)
    res = run_bass_kernel_spmd(nc, in_maps, core_ids=list(range(8)), trace=trace)
    B, T, C = x.shape
    out = np.empty((B, T, C), np.float32)
    for b in range(B):
        out[b] = (res.results[2 * b]["out"] + res.results[2 * b + 1]["out"]
                  + b_proj[None, :])
    return out, res


def kernel(x, W_attn, b_attn, W_proj, b_proj):
    out, _ = run_on_cores(x, W_attn, b_attn, W_proj, b_proj, trace=False)
    return out


# revision 10
# speedup vs baseline: 1.2308x; 1.2308x over previous
"""Causal self-attention (B=4, T=2048, C=1024, H=16) on 8 trn2 NeuronCores.

Sharding: core c -> batch b = c//2, heads h0 = (c%2)*8 .. h0+8 (tensor
parallel over heads: c_attn columns / c_proj rows split). Each core computes a
partial projection output [T, C]; the host sums the two partials per batch and
adds b_proj.

Device-side dataflow (bf16 matmul data, fp32 accumulation):
  - host passes x[b] pre-transposed as xt [C, T]
  - qT [C_head, T] computed with W_q column-slices stationary; bias bq added
    on DVE. kT has NO bias: softmax over keys is invariant to q.bk and bq.bk
    terms, so only (q+bq).k is needed -> k = x@W_k raw (saves ScalarE work)
  - v in natural [T, D] layout + a ones column so the PV matmul also produces
    the softmax denominator (row 64 of the PV accumulator)
  - scores: per head PAIR (2m, 2m+1), ROW-TILED 64-contraction matmuls at
    tile_position (0,0)/(64,0) run concurrently (~2x PE throughput); the
    sibling partition halves of kT/qT hold the two heads, no zero padding
  - attention runs per (pair, q-phase of 1024): PSUM = 2 x psS[128,1024]
    + 2 x psY[65,1024] = exactly 8 banks
  - P~ = exp(S^T/8) on ScalarE (the ONLY ScalarE work; scores are O(1) so no
    max-subtraction); diagonal 128x128 blocks masked with upper-tri 0/1 mask
  - normalize: DVE reciprocal of PSUM row 64 -> gpsimd partition_broadcast ->
    DVE multiply writing bf16 yT (no DRAM round-trips)
  - proj: bf16 stationary yT tiles (FWL) x bf16 W_proj rows, PSUM -> SBUF via
    DVE, DMA out on sync/gpsimd queues
"""

import numpy as np

P = 128


def _bf16_np():
    import ml_dtypes
    return ml_dtypes.bfloat16


def build_program(T=2048, C=1024, HC=8, D=64, num_devices=8, trn="TRN2"):
    import concourse.mybir as mybir
    import concourse.tile as tile
    from concourse import bacc
    from concourse.masks import make_upper_triangular

    W = 512          # matmul moving-dim chunk (psum-bank limit for fp32 out)
    PH = 1024        # attention q-phase width
    KC = C // P      # contraction tiles over C
    CO = HC * D      # this core's qkv channel block (512)
    NP = CO // P     # head pairs (2 heads of 64 = 1 partition tile)
    TT = T // P      # T tiles
    NCH = T // W     # T chunks
    MV = D + 1       # PV stationary columns: v + ones
    WO = 512         # proj output column chunk
    NW = C // WO
    dt32 = mybir.dt.float32
    bf16 = mybir.dt.bfloat16
    ActF = mybir.ActivationFunctionType
    Alu = mybir.AluOpType
    scale = 1.0 / float(np.sqrt(D))

    nc = bacc.Bacc(trn, target_bir_lowering=False, debug=False,
                   enable_asserts=False, num_devices=num_devices)

    xt_d = nc.dram_tensor("xt", [C, T], bf16, kind="ExternalInput")
    wq_d = nc.dram_tensor("wq", [C, CO], bf16, kind="ExternalInput")
    wk_d = nc.dram_tensor("wk", [C, CO], bf16, kind="ExternalInput")
    wv_d = nc.dram_tensor("wv", [C, CO], bf16, kind="ExternalInput")
    bq_d = nc.dram_tensor("bq", [P, NP], dt32, kind="ExternalInput")
    bvb_d = nc.dram_tensor("bvb", [P, CO], dt32, kind="ExternalInput")
    ones_d = nc.dram_tensor("ones", [P, TT * HC], bf16, kind="ExternalInput")
    wp_d = nc.dram_tensor("wp", [CO, C], bf16, kind="ExternalInput")
    out_d = nc.dram_tensor("out", [T, C], dt32, kind="ExternalOutput")
    lsc_d = [nc.dram_tensor(f"lsc{i}", [T], dt32) for i in range(2)]
    lsc2_d = [nc.dram_tensor(f"lsc2{i}", [T], dt32) for i in range(2)]

    with tile.TileContext(nc) as tc:
        with tc.tile_pool(name="const", bufs=1) as cpool, \
             tc.tile_pool(name="pers", bufs=1) as pers:
            tri = cpool.tile([P, P], bf16)
            make_upper_triangular(nc, tri[:], val=1.0, diag=True)
            bq_sb = cpool.tile([P, NP], dt32)
            nc.sync.dma_start(bq_sb[:], bq_d.ap())
            bvb_sb = cpool.tile([P, CO], dt32)
            nc.sync.dma_start(bvb_sb[:], bvb_d.ap())

            qT = pers.tile([P, NP, T], bf16, tag="qT")
            kT = pers.tile([P, HC, T], bf16, tag="kT")
            vaug = pers.tile([P, TT, HC, MV], bf16, tag="vaug")
            nc.sync.dma_start(
                vaug[:, :, :, D],
                ones_d.ap().rearrange("p (a b) -> p a b", b=HC))

            # ---------------- stage B: qkv projections ----------------
            with nc.named_scope("qkv"), \
                 tc.tile_pool(name="xtp", bufs=KC * NCH) as xpool, \
                 tc.tile_pool(name="wp_in", bufs=KC) as wpool, \
                 tc.tile_pool(name="psB", bufs=2, space="PSUM") as psB:
                xt_view = xt_d.ap().rearrange("(kc p) t -> kc p t", p=P)
                wq_view = wq_d.ap().rearrange("(kc p) n -> kc p n", p=P)
                xts = []
                wq_t = []
                dmae = [nc.sync, nc.gpsimd]
                di = 0
                for kc in range(KC):
                    wt = wpool.tile([P, CO], bf16, tag="w")
                    dmae[di % 2].dma_start(wt[:], wq_view[kc])
                    di += 1
                    wq_t.append(wt)
                    row = []
                    for cg in range(NCH):
                        xtc = xpool.tile([P, W], bf16, tag="xt")
                        dmae[di % 2].dma_start(
                            xtc[:], xt_view[kc][:, cg * W:(cg + 1) * W])
                        di += 1
                        row.append(xtc)
                    xts.append(row)

                def qk_stage(w_tiles, write_out):
                    for half in range((NP + 1) // 2):
                        ms = [m for m in (2 * half, 2 * half + 1) if m < NP]
                        pss = {}
                        for m in ms:
                            ps_m = psB.tile([P, T], dt32, tag="psB")
                            pss[m] = ps_m
                        for kc in range(KC):
                            for m in ms:
                                for cg in range(NCH):
                                    nc.tensor.matmul(
                                        pss[m][:, cg * W:(cg + 1) * W],
                                        w_tiles[kc][:, m * P:(m + 1) * P],
                                        xts[kc][cg][:],
                                        start=(kc == 0), stop=(kc == KC - 1),
                                        skip_group_check=True)
                        for m in ms:
                            write_out(m, pss[m])

                def write_qT(m, ps):
                    nc.vector.tensor_scalar_add(
                        qT[:, m, :], ps[:], bq_sb[:, m:m + 1])

                def write_kT(m, ps):
                    # unpacked per-head halves (row-tiled scores never read
                    # the sibling half, so no zero fill needed)
                    nc.vector.tensor_copy(kT[0:D, 2 * m, :], ps[0:D, :])
                    nc.vector.tensor_copy(kT[D:P, 2 * m + 1, :], ps[D:P, :])

                def load_w(w_d):
                    view = w_d.ap().rearrange("(kc p) n -> kc p n", p=P)
                    tiles = []
                    for kc in range(KC):
                        wt = wpool.tile([P, CO], bf16, tag="w")
                        dmae[kc % 2].dma_start(wt[:], view[kc])
                        tiles.append(wt)
                    return tiles

                qk_stage(wq_t, write_qT)
                qk_stage(load_w(wk_d), write_kT)

                wv_t = load_w(wv_d)
                bvb_v = bvb_sb[:].rearrange("p (h d) -> p h d", d=D)
                for tt in range(TT):
                    ps = psB.tile([P, CO], dt32, tag="psB")
                    for kc in range(KC):
                        nc.tensor.matmul(
                            ps[:],
                            xts[kc][tt * P // W][:, (tt * P % W):(tt * P % W) + P],
                            wv_t[kc][:],
                            start=(kc == 0), stop=(kc == KC - 1))
                    nc.vector.scalar_tensor_tensor(
                        out=vaug[:, tt, :, 0:D],
                        in0=ps[:].rearrange("p (h d) -> p h d", d=D),
                        scalar=1.0, in1=bvb_v,
                        op0=Alu.mult, op1=Alu.add)

            # ---------------- stage C: attention per head pair ----------------
            late_cm = tc.tile_pool(name="late", bufs=1)
            late = late_cm.__enter__()
            yT = late.tile([P, NP, T], bf16, tag="yT")
            with nc.named_scope("attn"), \
                 tc.tile_pool(name="ptp", bufs=6) as ptpool, \
                 tc.tile_pool(name="nrm", bufs=2) as nrmpool, \
                 tc.tile_pool(name="ysp", bufs=2) as yspool, \
                 tc.tile_pool(name="psS", bufs=2, space="PSUM") as psS, \
                 tc.tile_pool(name="psY", bufs=2, space="PSUM") as psY:

                def emit_s(h, ti, m, j, q0):
                    """Row-tiled scores for head h (row half ti) + exp -> pt."""
                    jb = j * P
                    qs0 = max(jb, q0)
                    qe_ph = q0 + PH
                    rows = slice(ti * D, ti * D + D)
                    sps = psS.tile([P, PH], dt32, tag="s")
                    for cg in range(qs0 // W, qe_ph // W):
                        qs = max(W * cg, qs0)
                        w = W * (cg + 1) - qs
                        nc.tensor.matmul(
                            sps[:, qs - q0:qs - q0 + w],
                            kT[rows, h, jb:jb + P],
                            qT[rows, m, qs:qs + w],
                            start=True, stop=True,
                            skip_group_check=True)
                    pt = ptpool.tile([P, qe_ph - qs0], bf16, tag="pt")
                    nc.scalar.activation(
                        pt[:], sps[:, qs0 - q0:PH], ActF.Exp, scale=scale)
                    if jb >= q0:  # diagonal block: mask upper triangle
                        nc.vector.tensor_mul(pt[:, 0:P], pt[:, 0:P], tri[:])
                    return pt

                def emit_pv(yt, h, j, q0, pt):
                    jb = j * P
                    qs0 = max(jb, q0)
                    for cg in range(qs0 // W, (q0 + PH) // W):
                        qs = max(W * cg, qs0)
                        w = W * (cg + 1) - qs
                        last_j = (W * (cg + 1)) // P - 1
                        nc.tensor.matmul(
                            yt[:, qs - q0:qs - q0 + w],
                            vaug[:, j, h, :],
                            pt[:, qs - qs0:qs - qs0 + w],
                            start=(j == 0), stop=(j == last_j),
                            skip_group_check=True)

                def finish(yt, h, m, q0):
                    # copy the PSUM accumulator out quickly (releases it for
                    # the next phase); the slow normalize chain runs
                    # SBUF-side via a DRAM round-trip that reshapes the
                    # denominator row into [128, 8] (DVE reciprocal is slow
                    # per element, so keep elements/lane tiny).
                    r0 = (h % 2) * D
                    lsc = lsc_d[h % 2]
                    lsc2 = lsc2_d[h % 2]
                    ys = yspool.tile([MV, PH], dt32, tag="ys")
                    nc.vector.tensor_copy(ys[:], yt[:])
                    nc.sync.dma_start(
                        lsc.ap()[q0:q0 + PH].rearrange("(o t) -> o t", o=1),
                        ys[D:D + 1, :])
                    l128 = nrmpool.tile([P, PH // P], dt32, tag="l128")
                    nc.gpsimd.dma_start(
                        l128[:],
                        lsc.ap()[q0:q0 + PH].rearrange("(p c) -> p c", p=P))
                    nc.vector.reciprocal(l128[:], l128[:])
                    nc.gpsimd.dma_start(
                        lsc2.ap()[q0:q0 + PH].rearrange("(p c) -> p c", p=P),
                        l128[:])
                    bc = nrmpool.tile([D, PH], dt32, tag="bc")
                    nc.sync.dma_start(
                        bc[:],
                        lsc2.ap()[q0:q0 + PH].rearrange(
                            "(o t) -> o t", o=1).broadcast_to([D, PH]))
                    nc.vector.tensor_mul(
                        yT[r0:r0 + D, m, q0:q0 + PH], ys[0:D, :], bc[:])

                for m in range(NP):
                    heads = (2 * m, 2 * m + 1)
                    for q0 in (0, PH):
                        js = range((q0 + PH) // P)
                        yts = {}
                        for ti, h in enumerate(heads):
                            yt_new = psY.tile([MV, PH], dt32, tag="yt")
                            yts[h] = yt_new
                        pending = []
                        for j in js:
                            for ti, h in enumerate(heads):
                                pt = emit_s(h, ti, m, j, q0)
                                pending.append((h, j, pt))
                                if len(pending) > 2:
                                    ph, pj, ppt = pending.pop(0)
                                    emit_pv(yts[ph], ph, pj, q0, ppt)
                        for ph, pj, ppt in pending:
                            emit_pv(yts[ph], ph, pj, q0, ppt)
                        for ti, h in enumerate(heads):
                            finish(yts[h], h, m, q0)

            # ---------------- stage E: output projection ----------------
            with nc.named_scope("proj"), \
                 tc.tile_pool(name="wpp", bufs=1) as wppool, \
                 tc.tile_pool(name="ost", bufs=3) as opool, \
                 tc.tile_pool(name="psO", bufs=4, space="PSUM") as psO:
                wpsb = wppool.tile([P, NP, C], bf16)
                nc.sync.dma_start(
                    wpsb[:], wp_d.ap().rearrange("(kt p) n -> p kt n", p=P))
                for tt in range(TT):
                    po = psO.tile([P, C], dt32, tag="o")
                    for kt in range(NP):
                        for nn in range(NW):
                            nc.tensor.matmul(
                                po[:, nn * WO:(nn + 1) * WO],
                                yT[:, kt, tt * P:(tt + 1) * P],
                                wpsb[:, kt, nn * WO:(nn + 1) * WO],
                                start=(kt == 0), stop=(kt == NP - 1),
                                skip_group_check=True)
                    ot = opool.tile([P, C], dt32, tag="ot")
                    nc.vector.tensor_copy(ot[:], po[:])
                    [nc.sync, nc.gpsimd][tt % 2].dma_start(
                        out_d.ap()[tt * P:(tt + 1) * P, :], ot[:])
            late_cm.__exit__(None, None, None)

    nc.compile()
    return nc


def make_core_inputs(x, W_attn, b_attn, W_proj, n_cores=8, HC=8, D=64):
    """Host-side sharding: per-core input dicts."""
    B, T, C = x.shape
    CO = HC * D
    NP = CO // P
    bf = _bf16_np()
    in_maps = []
    for c in range(n_cores):
        b = c // (n_cores // B)
        h0 = (c % (n_cores // B)) * HC
        lo = h0 * D
        bq = b_attn[lo:lo + CO]
        bv = b_attn[2 * C + lo:2 * C + lo + CO]
        in_maps.append({
            "xt": np.ascontiguousarray(x[b].T).astype(bf),
            "wq": np.ascontiguousarray(W_attn[:, lo:lo + CO]).astype(bf),
            "wk": np.ascontiguousarray(W_attn[:, C + lo:C + lo + CO]).astype(bf),
            "wv": np.ascontiguousarray(W_attn[:, 2 * C + lo:2 * C + lo + CO]).astype(bf),
            "bq": np.ascontiguousarray(bq.reshape(NP, P).T),
            "bvb": np.tile(bv[None, :], (P, 1)),
            "ones": np.ones((P, (T // P) * HC), bf),
            "wp": np.ascontiguousarray(W_proj[lo:lo + CO, :]).astype(bf),
        })
    return in_maps


_CACHE = {}


def _get_program():
    if "nc" not in _CACHE:
        _CACHE["nc"] = build_program()
    return _CACHE["nc"]


def run_on_cores(x, W_attn, b_attn, W_proj, b_proj, trace=False):
    """Returns (full output [B,T,C], BassKernelResults)."""
    from concourse.bass_utils import run_bass_kernel_spmd

    x = np.asarray(x, np.float32)
    W_attn = np.asarray(W_attn, np.float32)
    b_attn = np.asarray(b_attn, np.float32)
    W_proj = np.asarray(W_proj, np.float32)
    b_proj = np.asarray(b_proj, np.float32)

    nc = _get_program()
    in_maps = make_core_inputs(x, W_attn, b_attn, W_proj)
    res = run_bass_kernel_spmd(nc, in_maps, core_ids=list(range(8)), trace=trace)
    B, T, C = x.shape
    out = np.empty((B, T, C), np.float32)
    for b in range(B):
        out[b] = (res.results[2 * b]["out"] + res.results[2 * b + 1]["out"]
                  + b_proj[None, :])
    return out, res


def kernel(x, W_attn, b_attn, W_proj, b_proj):
    out, _ = run_on_cores(x, W_attn, b_attn, W_proj, b_proj, trace=False)
    return out


# revision 14
# speedup vs baseline: 1.2534x; 1.0184x over previous
"""Causal self-attention (B=4, T=2048, C=1024, H=16) on 8 trn2 NeuronCores.

Sharding: core c -> batch b = c//2, heads h0 = (c%2)*8 .. h0+8 (tensor
parallel over heads: c_attn columns / c_proj rows split). Each core computes a
partial projection output [T, C]; the host sums the two partials per batch and
adds b_proj.

Key design points (bf16 matmul data, fp32 accumulation):
  - kT has NO bias: softmax over keys is invariant to the q.bk and bq.bk
    terms, so only (q+bq).k is needed (bias bq added on DVE)
  - scores are ROW-TILED 64-contraction matmuls (tile_position (0,0)/(64,0)
    for the sibling partition halves holding heads 2m/2m+1) -> 2 cols/cycle
  - attention runs in 512-wide q-windows; the scores for TWO k-tiles (j,j+1)
    of one head land in one [128,2,512] PSUM tile so a single ScalarE Exp
    call covers both (ScalarE is the serial bottleneck; exp is its only job)
  - v is augmented with a ones column so the PV matmul also produces the
    softmax denominator (row 64 of the [65, 512] PV accumulator)
  - PSUM: psS 2x[128,2,512] (4 banks) + psY 2x[65,512] (2 banks) +
    psF 2x[128,512] (2 banks) = exactly 8 banks
  - ALL qkv / v / proj matmul work streams through the psF "filler" pool
    interleaved into the attention loop, keeping TensorE dense (HAM stays
    at 2.4 GHz) and hiding that work under the ScalarE exp stream
  - normalize: fast PSUM eviction, then a DRAM round-trip reshapes the
    denominator row to [128,4] (DVE reciprocal is slow per element), DMA
    broadcast back, DVE multiply writing bf16 yT
"""

import numpy as np

P = 128


def _bf16_np():
    import ml_dtypes
    return ml_dtypes.bfloat16


def build_program(T=2048, C=1024, HC=8, D=64, num_devices=8, trn="TRN2"):
    import concourse.mybir as mybir
    import concourse.tile as tile
    from concourse import bacc
    from concourse.masks import make_upper_triangular

    W = 512          # matmul moving chunk / attention window width
    KC = C // P      # contraction tiles over C
    CO = HC * D      # this core's qkv channel block (512)
    NP = CO // P     # head pairs
    TT = T // P      # k tiles
    NCH = T // W     # T chunks of 512
    NWIN = T // W    # attention q-windows
    MV = D + 1       # PV stationary columns: v + ones
    dt32 = mybir.dt.float32
    bf16 = mybir.dt.bfloat16
    ActF = mybir.ActivationFunctionType
    Alu = mybir.AluOpType
    scale = 1.0 / float(np.sqrt(D))

    nc = bacc.Bacc(trn, target_bir_lowering=False, debug=False,
                   enable_asserts=False, num_devices=num_devices)

    xt_d = nc.dram_tensor("xt", [C, T], bf16, kind="ExternalInput")
    wq_d = nc.dram_tensor("wq", [C, CO], bf16, kind="ExternalInput")
    wk_d = nc.dram_tensor("wk", [C, CO], bf16, kind="ExternalInput")
    wv_d = nc.dram_tensor("wv", [C, CO], bf16, kind="ExternalInput")
    bq_d = nc.dram_tensor("bq", [P, NP], dt32, kind="ExternalInput")
    bvb_d = nc.dram_tensor("bvb", [P, CO], dt32, kind="ExternalInput")
    ones_d = nc.dram_tensor("ones", [P, TT * HC], bf16, kind="ExternalInput")
    wp_d = nc.dram_tensor("wp", [CO, C], bf16, kind="ExternalInput")
    out_d = nc.dram_tensor("out", [T, C], dt32, kind="ExternalOutput")
    lsc_d = [nc.dram_tensor(f"lsc{i}", [T], dt32) for i in range(2)]
    lsc2_d = [nc.dram_tensor(f"lsc2{i}", [T], dt32) for i in range(2)]

    with tile.TileContext(nc) as tc:
        with tc.tile_pool(name="const", bufs=1) as cpool, \
             tc.tile_pool(name="pers", bufs=1) as pers, \
             tc.tile_pool(name="xtp", bufs=KC * NCH) as xpool, \
             tc.tile_pool(name="wp_in", bufs=3 * KC) as wpool, \
             tc.tile_pool(name="wpp", bufs=1) as wppool, \
             tc.tile_pool(name="ptp", bufs=8) as ptpool, \
             tc.tile_pool(name="ysp", bufs=2) as yspool, \
             tc.tile_pool(name="nrm", bufs=2) as nrmpool, \
             tc.tile_pool(name="ost", bufs=3) as opool, \
             tc.tile_pool(name="psS", bufs=2, space="PSUM") as psS, \
             tc.tile_pool(name="psY", bufs=2, space="PSUM") as psY, \
             tc.tile_pool(name="psF", bufs=2, space="PSUM") as psF:
            tri = cpool.tile([P, P], bf16)
            make_upper_triangular(nc, tri[:], val=1.0, diag=True)
            bq_sb = cpool.tile([P, NP], dt32)
            nc.sync.dma_start(bq_sb[:], bq_d.ap())
            bvb_sb = cpool.tile([P, CO], dt32)
            nc.sync.dma_start(bvb_sb[:], bvb_d.ap())

            qT = pers.tile([P, NP, T], bf16, tag="qT")
            kT = pers.tile([P, HC, T], bf16, tag="kT")
            vaug = pers.tile([P, TT, HC, MV], bf16, tag="vaug")
            nc.sync.dma_start(
                vaug[:, :, :, D],
                ones_d.ap().rearrange("p (a b) -> p a b", b=HC))
            yT = pers.tile([P, NP, T], bf16, tag="yT")
            wpsb = wppool.tile([P, NP, C], bf16)
            nc.gpsimd.dma_start(
                wpsb[:], wp_d.ap().rearrange("(kt p) n -> p kt n", p=P))

            # ---- input loads: x chunks and all three weight sets ----
            dmae = [nc.sync, nc.gpsimd]
            di = 0
            xt_view = xt_d.ap().rearrange("(kc p) t -> kc p t", p=P)
            xts = []
            for kc in range(KC):
                row = []
                for cg in range(NCH):
                    xtc = xpool.tile([P, W], bf16, tag="xt")
                    dmae[di % 2].dma_start(
                        xtc[:], xt_view[kc][:, cg * W:(cg + 1) * W])
                    di += 1
                    row.append(xtc)
                xts.append(row)

            def load_w(w_d):
                view = w_d.ap().rearrange("(kc p) n -> kc p n", p=P)
                tiles = []
                for kc in range(KC):
                    wt = wpool.tile([P, CO], bf16, tag="w")
                    dmae[(kc + di) % 2].dma_start(wt[:], view[kc])
                    tiles.append(wt)
                return tiles

            wq_t = load_w(wq_d)
            wv_t = load_w(wv_d)
            wk_t = load_w(wk_d)
            bvb_v = bvb_sb[:].rearrange("p (h d) -> p h d", d=D)

            # ---- filler chunk emitters (all stream through psF) ----
            def qk_chunk(kind, m, cg):
                w_t = wq_t if kind == "q" else wk_t
                ps = psF.tile([P, W], dt32, tag="f")
                for kc in range(KC):
                    nc.tensor.matmul(
                        ps[:], w_t[kc][:, m * P:(m + 1) * P], xts[kc][cg][:],
                        start=(kc == 0), stop=(kc == KC - 1),
                        skip_group_check=True)
                lo = cg * W
                if kind == "q":
                    nc.vector.tensor_scalar_add(
                        qT[:, m, lo:lo + W], ps[:], bq_sb[:, m:m + 1])
                else:
                    nc.vector.tensor_copy(kT[0:D, 2 * m, lo:lo + W], ps[0:D, :])
                    nc.vector.tensor_copy(
                        kT[D:P, 2 * m + 1, lo:lo + W], ps[D:P, :])

            def v_chunk(tt):
                ps = psF.tile([P, CO], dt32, tag="f")
                cg = tt * P // W
                o = tt * P % W
                for kc in range(KC):
                    nc.tensor.matmul(
                        ps[:], xts[kc][cg][:, o:o + P], wv_t[kc][:],
                        start=(kc == 0), stop=(kc == KC - 1),
                        skip_group_check=True)
                nc.vector.scalar_tensor_tensor(
                    out=vaug[:, tt, :, 0:D],
                    in0=ps[:].rearrange("p (h d) -> p h d", d=D),
                    scalar=1.0, in1=bvb_v,
                    op0=Alu.mult, op1=Alu.add)

            def proj_chunk(tt, nn):
                po = psF.tile([P, W], dt32, tag="f")
                for kt in range(NP):
                    nc.tensor.matmul(
                        po[:], yT[:, kt, tt * P:(tt + 1) * P],
                        wpsb[:, kt, nn * W:(nn + 1) * W],
                        start=(kt == 0), stop=(kt == NP - 1),
                        skip_group_check=True)
                ot = opool.tile([P, W], dt32, tag="ot")
                nc.vector.tensor_copy(ot[:], po[:])
                dmae[(tt + nn) % 2].dma_start(
                    out_d.ap()[tt * P:(tt + 1) * P, nn * W:(nn + 1) * W],
                    ot[:])

            # serial head: q/k for pair 0 (everything else is filler)
            for cg in range(NCH):
                qk_chunk("q", 0, cg)
            for cg in range(NCH):
                qk_chunk("k", 0, cg)

            # filler queue with deadlines (global j-step index by which each
            # chunk must have been EMITTED -- Tile dataflow requires writers
            # before readers in program order). v tiles first (PV needs tile
            # tt at m0's step ~tt), q/k for pair m before step 40*m, first
            # projection half after every pair finished windows 0-1.
            fillers = [(tt, ("v", tt)) for tt in range(TT)]
            for m in range(1, NP):
                fillers += [(40 * m - 22 + 2 * cg, ("q", m, cg))
                            for cg in range(NCH)]
                fillers += [(40 * m - 12 + 2 * cg, ("k", m, cg))
                            for cg in range(NCH)]
            # pair 3 windows 0-1 end at step 120+12; spread proj over 132..156
            fillers += [(134 + (2 * tt + nn) * 22 // 16, ("p", tt, nn))
                        for tt in range(TT // 2) for nn in range(2)]
            fillers.sort(key=lambda f: f[0])

            def emit_filler(f):
                if f[0] == "v":
                    v_chunk(f[1])
                elif f[0] == "p":
                    proj_chunk(f[1], f[2])
                else:
                    qk_chunk(f[0], f[1], f[2])

            fstate = {"emitted": 0, "step": 0}

            def step_fillers(lookahead=6, cap=3):
                n = 0
                while fstate["emitted"] < len(fillers):
                    dl, spec = fillers[fstate["emitted"]]
                    if dl > fstate["step"] and (n >= cap
                                                or dl > fstate["step"]
                                                + lookahead):
                        break
                    emit_filler(spec)
                    fstate["emitted"] += 1
                    n += 1

            # ---------------- attention ----------------
            def emit_s_pair(h, ti, m, jp, q0):
                """Scores for k-tiles (jp, jp+1) of head h -> one psS tile."""
                rows = slice(ti * D, ti * D + D)
                sps = psS.tile([P, 2, W], dt32, tag="s")
                for sl, j in enumerate((jp, jp + 1)):
                    jb = j * P
                    qs0 = max(jb, q0)
                    nc.tensor.matmul(
                        sps[:, sl, qs0 - q0:W],
                        kT[rows, h, jb:jb + P],
                        qT[rows, m, qs0:q0 + W],
                        start=True, stop=True,
                        skip_group_check=True)
                return sps

            def emit_exp(sps, jp, q0):
                """Exp the pair tile; returns list of (j, pt, off)."""
                jb0, jb1 = jp * P, (jp + 1) * P
                out = []
                if jb1 < q0:  # both slots full-width: single call
                    pt = ptpool.tile([P, 2, W], bf16, tag="pt")
                    nc.scalar.activation(pt[:], sps[:, :, :], ActF.Exp,
                                         scale=scale)
                    out.append((jp, pt[:, 0, :], 0))
                    out.append((jp + 1, pt[:, 1, :], 0))
                else:  # diagonal-touching: per-slot partial calls
                    for sl, j in enumerate((jp, jp + 1)):
                        jb = j * P
                        off = max(jb, q0) - q0
                        pt = ptpool.tile([P, W - off], bf16, tag="pt1")
                        nc.scalar.activation(pt[:], sps[:, sl, off:W],
                                             ActF.Exp, scale=scale)
                        if jb >= q0:
                            nc.vector.tensor_mul(pt[:, 0:P], pt[:, 0:P],
                                                 tri[:])
                        out.append((j, pt[:], 0))
                return out

            def emit_pv(yt, h, j, q0, pt):
                jb = j * P
                qs0 = max(jb, q0)
                last_j = (q0 + W) // P - 1
                nc.tensor.matmul(
                    yt[:, qs0 - q0:W],
                    vaug[:, j, h, :],
                    pt[:, 0:W - (qs0 - q0)] if qs0 > q0 else pt[:],
                    start=(j == 0), stop=(j == last_j),
                    skip_group_check=True)

            def finish(yt, h, m, q0):
                r0 = (h % 2) * D
                lsc = lsc_d[h % 2]
                lsc2 = lsc2_d[h % 2]
                ys = yspool.tile([MV, W], dt32, tag="ys")
                nc.vector.tensor_copy(ys[:], yt[:])
                nc.sync.dma_start(
                    lsc.ap()[q0:q0 + W].rearrange("(o t) -> o t", o=1),
                    ys[D:D + 1, :])
                l128 = nrmpool.tile([P, W // P], dt32, tag="l128")
                nc.gpsimd.dma_start(
                    l128[:],
                    lsc.ap()[q0:q0 + W].rearrange("(p c) -> p c", p=P))
                nc.vector.reciprocal(l128[:], l128[:])
                nc.gpsimd.dma_start(
                    lsc2.ap()[q0:q0 + W].rearrange("(p c) -> p c", p=P),
                    l128[:])
                bc = nrmpool.tile([D, W], dt32, tag="bc")
                nc.sync.dma_start(
                    bc[:],
                    lsc2.ap()[q0:q0 + W].rearrange(
                        "(o t) -> o t", o=1).broadcast_to([D, W]))
                nc.vector.tensor_mul(
                    yT[r0:r0 + D, m, q0:q0 + W], ys[0:D, :], bc[:])

            with nc.named_scope("attn"):
                for m in range(NP):
                    heads = (2 * m, 2 * m + 1)
                    for wi in range(NWIN):
                        q0 = wi * W
                        njs = 4 * wi + 4
                        yts = []
                        for ti, h in enumerate(heads):
                            yt_new = psY.tile([MV, W], dt32, tag="yt")
                            yts.append(yt_new)
                        pending = []
                        for jp in range(0, njs, 2):
                            for ti, h in enumerate(heads):
                                sps = emit_s_pair(h, ti, m, jp, q0)
                                for (j, pt, off) in emit_exp(sps, jp, q0):
                                    pending.append((ti, h, j, pt))
                                while len(pending) > 4:
                                    pti, ph, pj, ppt = pending.pop(0)
                                    emit_pv(yts[pti], ph, pj, q0, ppt)
                            fstate["step"] += 2
                            step_fillers()
                        for pti, ph, pj, ppt in pending:
                            emit_pv(yts[pti], ph, pj, q0, ppt)
                        for ti, h in enumerate(heads):
                            finish(yts[ti], h, m, q0)

            # drain remaining fillers, then the second projection half
            # (which needs the last attention window)
            fstate["step"] = 10 ** 6
            step_fillers(cap=len(fillers))
            with nc.named_scope("proj"):
                for tt in range(TT // 2, TT):
                    for nn in range(2):
                        proj_chunk(tt, nn)

    nc.compile()
    return nc


def make_core_inputs(x, W_attn, b_attn, W_proj, n_cores=8, HC=8, D=64):
    """Host-side sharding: per-core input dicts."""
    B, T, C = x.shape
    CO = HC * D
    NP = CO // P
    bf = _bf16_np()
    in_maps = []
    for c in range(n_cores):
        b = c // (n_cores // B)
        h0 = (c % (n_cores // B)) * HC
        lo = h0 * D
        bq = b_attn[lo:lo + CO]
        bv = b_attn[2 * C + lo:2 * C + lo + CO]
        in_maps.append({
            "xt": np.ascontiguousarray(x[b].T).astype(bf),
            "wq": np.ascontiguousarray(W_attn[:, lo:lo + CO]).astype(bf),
            "wk": np.ascontiguousarray(W_attn[:, C + lo:C + lo + CO]).astype(bf),
            "wv": np.ascontiguousarray(W_attn[:, 2 * C + lo:2 * C + lo + CO]).astype(bf),
            "bq": np.ascontiguousarray(bq.reshape(NP, P).T),
            "bvb": np.tile(bv[None, :], (P, 1)),
            "ones": np.ones((P, (T // P) * HC), bf),
            "wp": np.ascontiguousarray(W_proj[lo:lo + CO, :]).astype(bf),
        })
    return in_maps


_CACHE = {}


def _get_program():
    if "nc" not in _CACHE:
        _CACHE["nc"] = build_program()
    return _CACHE["nc"]


def run_on_cores(x, W_attn, b_attn, W_proj, b_proj, trace=False):
    """Returns (full output [B,T,C], BassKernelResults)."""
    from concourse.bass_utils import run_bass_kernel_spmd

    x = np.asarray(x, np.float32)
    W_attn = np.asarray(W_attn, np.float32)
    b_attn = np.asarray(b_attn, np.float32)
    W_proj = np.asarray(W_proj, np.float32)
    b_proj = np.asarray(b_proj, np.float32)

    nc = _get_program()
    in_maps = make_core_inputs(x, W_attn, b_attn, W_proj)
    res = run_bass_kernel_spmd(nc, in_maps, core_ids=list(range(8)), trace=trace)
    B, T, C = x.shape
    out = np.empty((B, T, C), np.float32)
    for b in range(B):
        out[b] = (res.results[2 * b]["out"] + res.results[2 * b + 1]["out"]
                  + b_proj[None, :])
    return out, res


def kernel(x, W_attn, b_attn, W_proj, b_proj):
    out, _ = run_on_cores(x, W_attn, b_attn, W_proj, b_proj, trace=False)
    return out
